# revision 6
# baseline (speedup 1.0000x reference)
# Trainium2 Bass kernel for nn_AttentionLayer (BiDAF-style attention).
#
# Math (T=16384, J=1024, D2=512):
#   w1,w2,w3 = Ws blocks;  S[t,j] = H@w1 + U@w2 + (H*w3)@U.T
#   A  = softmax_j(S) @ U                      (C2Q)
#   b  = softmax_t(max_j S);  h~ = b @ H       (Q2C, global over T)
#   G  = [H | A | H*A | H*h~]                  (T, 2048)
#
# Sharding: T rows split across 8 cores (2048 rows each). U/Ws replicated.
# Per core everything is local except (hnum = sum_t exp(m_t) H_t, ssum =
# sum_t exp(m_t)) which are AllReduce'd (513 floats).
#
# Device-side layout trick: compute S^T tiles [j_part, t_free] so that the
# C2Q attend matmul (A = P @ U) can use E=exp(S^T) slices directly as the
# stationary operand.  exp bias handles the s2[j] term (per-partition), and
# the s1[t] term cancels in softmax_j; it is reapplied only to the Q2C row
# maxima (bnum = max_j E'' * exp(s1)).
#
# Matmuls run as float32r (TF32-like: measured ~1.5e-4 rel err, ~3x faster
# than plain fp32 on TRN2).

import numpy as np

T, J, D2 = 16384, 1024, 512
NCORES = 8
TC = T // NCORES            # 2048 context rows per core
NCHUNK = 4                  # t-chunks per core
CHUNK = TC // NCHUNK        # 512
NTT = TC // 128             # 16 t-tiles per core
NJT = J // 128              # 8 j-tiles
NKT = D2 // 128             # 4 d-tiles

_CACHE = {}
LAST = {}


def _build_nc():
    import concourse.bacc as bacc
    import concourse.mybir as mybir
    import concourse.tile as tile

    f32 = mybir.dt.float32
    f32r = mybir.dt.float32r
    X = mybir.AxisListType.X
    MAX = mybir.AluOpType.max
    ADD = mybir.AluOpType.add
    MULT = mybir.AluOpType.mult
    EXP = mybir.ActivationFunctionType.Exp
    COPY = mybir.ActivationFunctionType.Copy

    def f(ap):  # view an fp32r AP as plain fp32 for non-matmul consumers
        return ap.bitcast(f32)

    nc = bacc.Bacc("TRN2", target_bir_lowering=False, debug=False,
                   num_devices=NCORES)

    HT = nc.dram_tensor("HT", [D2, TC], f32r, kind="ExternalInput")
    Hn = nc.dram_tensor("Hn", [TC, D2], f32r, kind="ExternalInput")
    UT = nc.dram_tensor("UT", [D2, J], f32r, kind="ExternalInput")
    Un = nc.dram_tensor("Un", [J, D2], f32r, kind="ExternalInput")
    UW = nc.dram_tensor("UW", [D2, J], f32r, kind="ExternalInput")
    Wc = nc.dram_tensor("Wc", [D2, 4], f32r, kind="ExternalInput")
    Id = nc.dram_tensor("Id", [128, 128], f32, kind="ExternalInput")
    On = nc.dram_tensor("On", [1, 128], f32r, kind="ExternalInput")
    G = nc.dram_tensor("G", [TC, 4 * D2], f32, kind="ExternalOutput")

    with tile.TileContext(nc) as tc:
        with (
            tc.tile_pool(name="persist", bufs=1) as pp,
            tc.tile_pool(name="stream", bufs=2) as sp,
            tc.tile_pool(name="stage", bufs=4) as gp,
            tc.tile_pool(name="spsum", bufs=2, space="PSUM") as spsum,
            tc.tile_pool(name="apsum", bufs=2, space="PSUM") as apsum,
            tc.tile_pool(name="trpsum", bufs=2, space="PSUM") as trpsum,
            tc.tile_pool(name="rowpsum", bufs=1, space="PSUM") as rowpsum,
            tc.tile_pool(name="dram", bufs=1, space="DRAM") as dram,
        ):
            # ---- dummy collective first: pays the ~70us first-collective
            # warmup on TOPSP/SDMA while the engines do real work.
            dummy_sb = pp.tile([1, 8], f32, tag="dummy_sb")
            nc.vector.memset(dummy_sb[:], 0.0)
            dummy_in = dram.tile([1, 8], f32, tag="dummy_in")
            dummy_out = dram.tile([1, 8], f32, tag="dummy_out")
            nc.sync.dma_start(dummy_in[:], dummy_sb[:])
            nc.gpsimd.collective_compute(
                "AllReduce", ADD, replica_groups=[list(range(NCORES))],
                ins=[dummy_in.opt()], outs=[dummy_out.opt()],
            )

            # ---- persistent loads
            ident = pp.tile([128, 128], f32, tag="ident")
            nc.sync.dma_start(ident[:], Id.ap()[:])
            wc = pp.tile([128, NKT, 4], f32r, tag="wc")
            nc.sync.dma_start(wc[:], Wc.ap().rearrange("(kt p) w -> p kt w", p=128))
            ut = pp.tile([128, NKT, J], f32r, tag="ut")
            nc.sync.dma_start(ut[:], UT.ap().rearrange("(kt p) j -> p kt j", p=128))
            un = pp.tile([128, NJT, D2], f32r, tag="un")
            nc.sync.dma_start(un[:], Un.ap().rearrange("(jt p) d -> p jt d", p=128))
            ht = pp.tile([128, NKT, TC], f32r, tag="ht")
            hn = pp.tile([128, NTT, D2], f32r, tag="hn")
            for c in range(NCHUNK):
                cs, ce = c * CHUNK, (c + 1) * CHUNK
                nc.sync.dma_start(
                    ht[:, :, cs:ce],
                    HT.ap()[:, cs:ce].rearrange("(kt p) t -> p kt t", p=128))
                nc.sync.dma_start(
                    hn[:, 4 * c:4 * (c + 1), :],
                    Hn.ap()[cs:ce, :].rearrange("(tt p) d -> p tt d", p=128))

            # uw3[d, j] = U^T * w3[d]  (host-prescaled; stationary for S^T)
            uw3 = pp.tile([128, NKT, J], f32r, tag="uw3")
            nc.sync.dma_start(uw3[:], UW.ap().rearrange("(kt p) j -> p kt j", p=128))

            onesrow = pp.tile([1, 128], f32r, tag="onesrow")
            nc.sync.dma_start(onesrow[:], On.ap()[:])
            onescol = wc[:, 0, 3:4]

            # ---- s2[j] = U @ w2, as per-(j)-partition columns for exp bias
            s2row = pp.tile([1, J], f32, tag="s2row")
            for jc in range(J // 512):
                s2ps = rowpsum.tile([1, 512], f32, tag="row")
                for kt in range(NKT):
                    nc.tensor.matmul(s2ps[:], wc[:, kt, 1:2],
                                     ut[:, kt, jc * 512:(jc + 1) * 512],
                                     start=(kt == 0), stop=(kt == NKT - 1))
                nc.vector.tensor_copy(s2row[0:1, jc * 512:(jc + 1) * 512], s2ps[:])
            s2col = pp.tile([128, NJT], f32, tag="s2col")
            for jt in range(NJT):
                tp = trpsum.tile([128, 1], f32, tag="tr")
                nc.tensor.transpose(tp[:], s2row[0:1, jt * 128:(jt + 1) * 128],
                                    ident[0:1, 0:1])
                nc.vector.tensor_copy(s2col[:, jt:jt + 1], tp[:])

            # ---- persistent accumulators
            emax = pp.tile([128, NTT], f32, tag="emax")    # max_j E'' per t
            dcol = pp.tile([128, NTT], f32, tag="dcol")    # sum_j E'' per t
            es1 = pp.tile([128, NTT], f32, tag="es1")      # exp(s1[t])
            bnum = pp.tile([128, NTT], f32r, tag="bnum")  # exp(m[t])
            hnum_sb = pp.tile([1, D2], f32, tag="hnum_sb")  # sum_t bnum*H

            for c in range(NCHUNK):
                cs, ce = c * CHUNK, (c + 1) * CHUNK

                # S^T tiles for this chunk -> E'' = exp(S^T + s2[j])
                e = sp.tile([128, NJT, CHUNK], f32r, tag="e")
                for jt in range(NJT):
                    sps = spsum.tile([128, CHUNK], f32, tag="sps")
                    for kt in range(NKT):
                        nc.tensor.matmul(
                            sps[:],
                            uw3[:, kt, jt * 128:(jt + 1) * 128],
                            ht[:, kt, cs:ce],
                            start=(kt == 0), stop=(kt == NKT - 1))
                    nc.scalar.activation(e[:, jt, :], sps[:], EXP,
                                         bias=s2col[:, jt:jt + 1])

                # partial reductions over the 8 j-tiles (partition-slot-wise)
                pmax = sp.tile([128, CHUNK], f32, tag="pmax")
                psm = sp.tile([128, CHUNK], f32, tag="psm")
                ev = f(e[:]).rearrange("p jt t -> p t jt")
                nc.vector.tensor_reduce(pmax[:], ev, X, MAX)
                nc.vector.tensor_reduce(psm[:], ev, X, ADD)

                # s1[t] for this chunk (row form), then exp into column form
                s1ps = rowpsum.tile([1, CHUNK], f32, tag="row")
                for kt in range(NKT):
                    nc.tensor.matmul(s1ps[:], wc[:, kt, 0:1],
                                     ht[:, kt, cs:ce],
                                     start=(kt == 0), stop=(kt == NKT - 1))
                s1row = sp.tile([1, CHUNK], f32, tag="s1row")
                nc.vector.tensor_copy(s1row[:], s1ps[:])

                hnps = rowpsum.tile([1, D2], f32, tag="row")
                for i in range(4):
                    tt = 4 * c + i
                    # transpose partials 128-block -> reduce over partitions
                    tpm = trpsum.tile([128, 128], f32, tag="tr")
                    nc.tensor.transpose(tpm[:], pmax[:, i * 128:(i + 1) * 128],
                                        ident[:])
                    nc.vector.tensor_reduce(emax[:, tt:tt + 1], tpm[:], X, MAX)
                    tps = trpsum.tile([128, 128], f32, tag="tr")
                    nc.tensor.transpose(tps[:], psm[:, i * 128:(i + 1) * 128],
                                        ident[:])
                    nc.vector.tensor_reduce(dcol[:, tt:tt + 1], tps[:], X, ADD)
                    # s1 column + exp
                    ts1 = trpsum.tile([128, 1], f32, tag="tr")
                    nc.tensor.transpose(ts1[:], s1row[0:1, i * 128:(i + 1) * 128],
                                        ident[0:1, 0:1])
                    nc.scalar.activation(es1[:, tt:tt + 1], ts1[:], EXP)
                    # bnum = exp(m[t]) = emax * exp(s1)
                    nc.vector.tensor_tensor(bnum[:, tt:tt + 1],
                                            emax[:, tt:tt + 1],
                                            es1[:, tt:tt + 1], MULT)
                    # Q2C numerator: hnps += bnum_tile.T @ H_tile
                    nc.tensor.matmul(hnps[:], bnum[:, tt:tt + 1],
                                     hn[:, tt, :],
                                     start=(i == 0), stop=(i == 3))

                    # C2Q attend: A = (E''.T @ U) / D
                    aps = apsum.tile([128, D2], f32, tag="aps")
                    for jt in range(NJT):
                        nc.tensor.matmul(
                            aps[:],
                            e[:, jt, i * 128:(i + 1) * 128],
                            un[:, jt, :],
                            start=(jt == 0), stop=(jt == NJT - 1))
                    dinv = gp.tile([128, 1], f32, tag="dinv")
                    nc.vector.reciprocal(dinv[:], dcol[:, tt:tt + 1])
                    a_sb = gp.tile([128, D2], f32, tag="a_sb")
                    nc.vector.tensor_scalar_mul(a_sb[:], aps[:], dinv[:])
                    ha_sb = gp.tile([128, D2], f32, tag="ha_sb")
                    nc.vector.tensor_tensor(ha_sb[:], f(hn[:, tt, :]), a_sb[:], MULT)

                    ts_, te_ = tt * 128, (tt + 1) * 128
                    nc.sync.dma_start(G.ap()[ts_:te_, 0:D2], f(hn[:, tt, :]))
                    nc.sync.dma_start(G.ap()[ts_:te_, D2:2 * D2], a_sb[:])
                    nc.sync.dma_start(G.ap()[ts_:te_, 2 * D2:3 * D2], ha_sb[:])

                # fold chunk's Q2C numerator into SBUF accumulator
                if c == 0:
                    nc.vector.tensor_copy(hnum_sb[:], hnps[:])
                else:
                    nc.vector.tensor_tensor(hnum_sb[:], hnum_sb[:], hnps[:], ADD)

            # ---- Q2C global: AllReduce(hnum | ssum)
            ssps = rowpsum.tile([1, NTT], f32, tag="row")
            nc.tensor.matmul(ssps[:], onescol, bnum[:],
                             start=True, stop=True)
            arow = pp.tile([1, 520], f32, tag="arow")
            nc.vector.memset(arow[:], 0.0)
            nc.vector.tensor_copy(arow[0:1, 0:D2], hnum_sb[:])
            nc.vector.tensor_reduce(arow[0:1, D2:D2 + 1], ssps[:], X, ADD)
            ar_in = dram.tile([1, 520], f32, tag="ar_in")
            ar_out = dram.tile([1, 520], f32, tag="ar_out")
            nc.sync.dma_start(ar_in[:], arow[:])
            nc.gpsimd.collective_compute(
                "AllReduce", ADD, replica_groups=[list(range(NCORES))],
                ins=[ar_in.opt()], outs=[ar_out.opt()],
            )
            hg = pp.tile([1, 520], f32, tag="hg")
            nc.sync.dma_start(hg[:], ar_out[:])

            # h~ = hnum_g / ssum_g, broadcast to all partitions
            zinv = pp.tile([1, 1], f32, tag="zinv")
            nc.vector.reciprocal(zinv[:], hg[0:1, D2:D2 + 1])
            htrow = pp.tile([1, D2], f32r, tag="htrow")
            nc.vector.tensor_scalar_mul(htrow[:], hg[0:1, 0:D2], zinv[:])
            htps = rowpsum.tile([128, D2], f32, tag="bcast")
            nc.tensor.matmul(htps[:], onesrow[:], htrow[:],
                             start=True, stop=True)

            # G block 3: H * h~
            for tt in range(NTT):
                hh_sb = gp.tile([128, D2], f32, tag="hh_sb")
                nc.vector.tensor_tensor(hh_sb[:], f(hn[:, tt, :]), htps[:], MULT)
                nc.sync.dma_start(G.ap()[tt * 128:(tt + 1) * 128, 3 * D2:4 * D2],
                                  hh_sb[:])

    nc.compile()
    return nc


def kernel(H, U, Ws):
    from concourse import bass_utils

    H = np.ascontiguousarray(np.asarray(H, dtype=np.float32))
    U = np.ascontiguousarray(np.asarray(U, dtype=np.float32))
    Ws = np.asarray(Ws, dtype=np.float32)

    if "nc" not in _CACHE:
        _CACHE["nc"] = _build_nc()
    nc = _CACHE["nc"]

    UT = np.ascontiguousarray(U.T)
    w3 = Ws[2 * D2:3 * D2, 0]
    UW = np.ascontiguousarray(UT * w3[:, None])
    Wc = np.ones((D2, 4), dtype=np.float32)
    Wc[:, 0:3] = Ws.reshape(3, D2).T
    ident = np.eye(128, dtype=np.float32)

    in_maps = []
    for c in range(NCORES):
        Hc = H[c * TC:(c + 1) * TC]
        in_maps.append({
            "HT": np.ascontiguousarray(Hc.T),
            "Hn": Hc,
            "UT": UT,
            "Un": U,
            "UW": UW,
            "Wc": Wc,
            "Id": ident,
            "On": np.ones((1, 128), dtype=np.float32),
        })

    res = bass_utils.run_bass_kernel_spmd(
        nc, in_maps, core_ids=list(range(NCORES)))
    LAST["exec_time_ns"] = res.exec_time_ns
    G_full = np.concatenate([res.results[c]["G"] for c in range(NCORES)],
                            axis=0)
    return G_full.astype(np.float32, copy=False)


# revision 7
# speedup vs baseline: 1.4724x; 1.4724x over previous
# Trainium2 Bass kernel for nn_AttentionLayer (BiDAF-style attention).
#
# Math (T=16384, J=1024, D2=512):
#   w1,w2,w3 = Ws blocks;  S[t,j] = H@w1 + U@w2 + (H*w3)@U.T
#   A  = softmax_j(S) @ U                      (C2Q)
#   b  = softmax_t(max_j S);  h~ = b @ H       (Q2C, global over T)
#   G  = [H | A | H*A | H*h~]                  (T, 2048)
#
# Sharding: T rows split across 8 cores (2048 rows each). U/Ws replicated.
# Per core everything is local except (hnum = sum_t exp(m_t) H_t, ssum =
# sum_t exp(m_t)) which are AllReduce'd (513 floats).  A dummy AllReduce
# fires at kernel start to absorb the ~70us first-collective warmup.
#
# Layout trick: compute S^T tiles [j_part, t_free] so the C2Q attend matmul
# (A = P @ U) can use E=exp(S^T) slices directly as the stationary operand.
# exp bias handles the s2[j] term (per-partition); the s1[t] term cancels in
# softmax_j and is reapplied only to the Q2C row maxima.
#
# Dtypes: S/A matmuls in bf16 (PE-bound kernel; halves weight-load + stream
# cost), Q2C path (hnum, htile) in float32r (TF32-like).

import numpy as np

T, J, D2 = 16384, 1024, 512
NCORES = 8
TC = T // NCORES            # 2048 context rows per core
NCHUNK = 4                  # t-chunks per core
CHUNK = TC // NCHUNK        # 512
NTT = TC // 128             # 16 t-tiles per core
NJT = J // 128              # 8 j-tiles
NKT = D2 // 128             # 4 d-tiles

MM_BF16 = True              # bf16 for the S / A / s1 / s2 matmul operands

_CACHE = {}
LAST = {}


def _build_nc():
    import concourse.bacc as bacc
    import concourse.mybir as mybir
    import concourse.tile as tile

    f32 = mybir.dt.float32
    f32r = mybir.dt.float32r
    bf16 = mybir.dt.bfloat16
    mmdt = bf16 if MM_BF16 else f32r
    X = mybir.AxisListType.X
    MAX = mybir.AluOpType.max
    ADD = mybir.AluOpType.add
    MULT = mybir.AluOpType.mult
    EXP = mybir.ActivationFunctionType.Exp

    def f(ap):  # view an fp32r AP as plain fp32 for non-matmul consumers
        return ap.bitcast(f32)

    nc = bacc.Bacc("TRN2", target_bir_lowering=False, debug=False,
                   num_devices=NCORES)

    HT = nc.dram_tensor("HT", [D2, TC], mmdt, kind="ExternalInput")
    Hn = nc.dram_tensor("Hn", [TC, D2], f32r, kind="ExternalInput")
    UT = nc.dram_tensor("UT", [D2, J], mmdt, kind="ExternalInput")
    Un = nc.dram_tensor("Un", [J, D2], mmdt, kind="ExternalInput")
    UW = nc.dram_tensor("UW", [D2, J], mmdt, kind="ExternalInput")
    Wc = nc.dram_tensor("Wc", [D2, 2], mmdt, kind="ExternalInput")
    Id = nc.dram_tensor("Id", [128, 128], f32, kind="ExternalInput")
    On = nc.dram_tensor("On", [1, 128], f32r, kind="ExternalInput")
    Oc = nc.dram_tensor("Oc", [128, 1], f32r, kind="ExternalInput")
    G = nc.dram_tensor("G", [TC, 4 * D2], f32, kind="ExternalOutput")

    with tile.TileContext(nc) as tc:
        with (
            tc.tile_pool(name="persist", bufs=1) as pp,
            tc.tile_pool(name="stream", bufs=2) as sp,
            tc.tile_pool(name="stage", bufs=4) as gp,
            tc.tile_pool(name="spsum", bufs=2, space="PSUM") as spsum,
            tc.tile_pool(name="apsum", bufs=2, space="PSUM") as apsum,
            tc.tile_pool(name="trpsum", bufs=2, space="PSUM") as trpsum,
            tc.tile_pool(name="rowpsum", bufs=1, space="PSUM") as rowpsum,
            tc.tile_pool(name="dram", bufs=1, space="DRAM") as dram,
        ):
            # ---- dummy collective first: pays the ~70us first-collective
            # warmup on TOPSP/SDMA while the engines do real work.
            dummy_sb = pp.tile([1, 8], f32, tag="dummy_sb")
            nc.vector.memset(dummy_sb[:], 0.0)
            dummy_in = dram.tile([1, 8], f32, tag="dummy_in")
            dummy_out = dram.tile([1, 8], f32, tag="dummy_out")
            nc.sync.dma_start(dummy_in[:], dummy_sb[:])
            nc.gpsimd.collective_compute(
                "AllReduce", ADD, replica_groups=[list(range(NCORES))],
                ins=[dummy_in.opt()], outs=[dummy_out.opt()],
            )

            # ---- persistent loads
            ident = pp.tile([128, 128], f32, tag="ident")
            nc.sync.dma_start(ident[:], Id.ap()[:])
            wc = pp.tile([128, NKT, 2], mmdt, tag="wc")
            nc.sync.dma_start(wc[:], Wc.ap().rearrange("(kt p) w -> p kt w", p=128))
            ut = pp.tile([128, NKT, J], mmdt, tag="ut")
            nc.sync.dma_start(ut[:], UT.ap().rearrange("(kt p) j -> p kt j", p=128))
            un = pp.tile([128, NJT, D2], mmdt, tag="un")
            nc.sync.dma_start(un[:], Un.ap().rearrange("(jt p) d -> p jt d", p=128))
            uw3 = pp.tile([128, NKT, J], mmdt, tag="uw3")
            nc.sync.dma_start(uw3[:], UW.ap().rearrange("(kt p) j -> p kt j", p=128))
            ht = pp.tile([128, NKT, TC], mmdt, tag="ht")
            hn = pp.tile([128, NTT, D2], f32r, tag="hn")
            for c in range(NCHUNK):
                cs, ce = c * CHUNK, (c + 1) * CHUNK
                nc.sync.dma_start(
                    ht[:, :, cs:ce],
                    HT.ap()[:, cs:ce].rearrange("(kt p) t -> p kt t", p=128))
                nc.sync.dma_start(
                    hn[:, 4 * c:4 * (c + 1), :],
                    Hn.ap()[cs:ce, :].rearrange("(tt p) d -> p tt d", p=128))

            onesrow = pp.tile([1, 128], f32r, tag="onesrow")
            nc.sync.dma_start(onesrow[:], On.ap()[:])
            onescol = pp.tile([128, 1], f32r, tag="onescol")
            nc.sync.dma_start(onescol[:], Oc.ap()[:])

            # ---- s2[j] = U @ w2, as per-(j)-partition columns for exp bias
            s2row = pp.tile([1, J], f32, tag="s2row")
            for jc in range(J // 512):
                s2ps = rowpsum.tile([1, 512], f32, tag="row")
                for kt in range(NKT):
                    nc.tensor.matmul(s2ps[:], wc[:, kt, 1:2],
                                     ut[:, kt, jc * 512:(jc + 1) * 512],
                                     start=(kt == 0), stop=(kt == NKT - 1))
                nc.vector.tensor_copy(s2row[0:1, jc * 512:(jc + 1) * 512], s2ps[:])
            s2col = pp.tile([128, NJT], f32, tag="s2col")
            for jt in range(NJT):
                tp = trpsum.tile([128, 1], f32, tag="tr")
                nc.tensor.transpose(tp[:], s2row[0:1, jt * 128:(jt + 1) * 128],
                                    ident[0:1, 0:1])
                nc.vector.tensor_copy(s2col[:, jt:jt + 1], tp[:])

            # ---- persistent accumulators
            emax = pp.tile([128, NTT], f32, tag="emax")    # max_j E'' per t
            dcol = pp.tile([128, NTT], f32, tag="dcol")    # sum_j E'' per t
            es1 = pp.tile([128, NTT], f32, tag="es1")      # exp(s1[t])
            bnum = pp.tile([128, NTT], f32r, tag="bnum")   # exp(m[t])
            hnum_sb = pp.tile([1, D2], f32, tag="hnum_sb")  # sum_t bnum*H

            for c in range(NCHUNK):
                cs, ce = c * CHUNK, (c + 1) * CHUNK

                # S^T tiles for this chunk -> E'' = exp(S^T + s2[j])
                e = sp.tile([128, NJT, CHUNK], mmdt, tag="e")
                for jt in range(NJT):
                    sps = spsum.tile([128, CHUNK], f32, tag="sps")
                    for kt in range(NKT):
                        nc.tensor.matmul(
                            sps[:],
                            uw3[:, kt, jt * 128:(jt + 1) * 128],
                            ht[:, kt, cs:ce],
                            start=(kt == 0), stop=(kt == NKT - 1))
                    nc.scalar.activation(e[:, jt, :], sps[:], EXP,
                                         bias=s2col[:, jt:jt + 1])

                # partial max/sum over the 8 j-tiles (chained tensor_tensor;
                # strided tensor_reduce was ~5x slower on DVE)
                pmax = sp.tile([128, CHUNK], f32, tag="pmax")
                psm = sp.tile([128, CHUNK], f32, tag="psm")
                nc.vector.tensor_tensor(pmax[:], e[:, 0, :], e[:, 1, :], MAX)
                nc.vector.tensor_tensor(psm[:], e[:, 0, :], e[:, 1, :], ADD)
                for jt in range(2, NJT):
                    nc.vector.tensor_tensor(pmax[:], pmax[:], e[:, jt, :], MAX)
                    nc.vector.tensor_tensor(psm[:], psm[:], e[:, jt, :], ADD)

                # s1[t] for this chunk (row form), then exp into column form
                s1ps = rowpsum.tile([1, CHUNK], f32, tag="row")
                for kt in range(NKT):
                    nc.tensor.matmul(s1ps[:], wc[:, kt, 0:1],
                                     ht[:, kt, cs:ce],
                                     start=(kt == 0), stop=(kt == NKT - 1))
                s1row = sp.tile([1, CHUNK], f32, tag="s1row")
                nc.vector.tensor_copy(s1row[:], s1ps[:])

                hnps = rowpsum.tile([1, D2], f32, tag="row")
                for i in range(4):
                    tt = 4 * c + i
                    # transpose partials 128-block -> reduce over partitions
                    tpm = trpsum.tile([128, 128], f32, tag="tr")
                    nc.tensor.transpose(tpm[:], pmax[:, i * 128:(i + 1) * 128],
                                        ident[:])
                    nc.vector.tensor_reduce(emax[:, tt:tt + 1], tpm[:], X, MAX)
                    tps = trpsum.tile([128, 128], f32, tag="tr")
                    nc.tensor.transpose(tps[:], psm[:, i * 128:(i + 1) * 128],
                                        ident[:])
                    nc.vector.tensor_reduce(dcol[:, tt:tt + 1], tps[:], X, ADD)
                    # s1 column + exp
                    ts1 = trpsum.tile([128, 1], f32, tag="tr")
                    nc.tensor.transpose(ts1[:], s1row[0:1, i * 128:(i + 1) * 128],
                                        ident[0:1, 0:1])
                    nc.scalar.activation(es1[:, tt:tt + 1], ts1[:], EXP)
                    # bnum = exp(m[t]) = emax * exp(s1)
                    nc.vector.tensor_tensor(bnum[:, tt:tt + 1],
                                            emax[:, tt:tt + 1],
                                            es1[:, tt:tt + 1], MULT)
                    # Q2C numerator: hnps += bnum_tile.T @ H_tile
                    nc.tensor.matmul(hnps[:], bnum[:, tt:tt + 1],
                                     hn[:, tt, :],
                                     start=(i == 0), stop=(i == 3))

                    # C2Q attend: A = (E''.T @ U) / D
                    aps = apsum.tile([128, D2], f32, tag="aps")
                    for jt in range(NJT):
                        nc.tensor.matmul(
                            aps[:],
                            e[:, jt, i * 128:(i + 1) * 128],
                            un[:, jt, :],
                            start=(jt == 0), stop=(jt == NJT - 1))
                    dinv = gp.tile([128, 1], f32, tag="dinv")
                    nc.vector.reciprocal(dinv[:], dcol[:, tt:tt + 1])
                    a_sb = gp.tile([128, D2], f32, tag="a_sb")
                    nc.vector.tensor_scalar_mul(a_sb[:], aps[:], dinv[:])
                    ha_sb = gp.tile([128, D2], f32, tag="ha_sb")
                    nc.vector.tensor_tensor(ha_sb[:], f(hn[:, tt, :]), a_sb[:], MULT)

                    ts_, te_ = tt * 128, (tt + 1) * 128
                    nc.sync.dma_start(G.ap()[ts_:te_, 0:D2], f(hn[:, tt, :]))
                    nc.sync.dma_start(G.ap()[ts_:te_, D2:2 * D2], a_sb[:])
                    nc.sync.dma_start(G.ap()[ts_:te_, 2 * D2:3 * D2], ha_sb[:])

                # fold chunk's Q2C numerator into SBUF accumulator
                if c == 0:
                    nc.vector.tensor_copy(hnum_sb[:], hnps[:])
                else:
                    nc.vector.tensor_tensor(hnum_sb[:], hnum_sb[:], hnps[:], ADD)

            # ---- Q2C global: AllReduce(hnum | ssum)
            ssps = rowpsum.tile([1, NTT], f32, tag="row")
            nc.tensor.matmul(ssps[:], onescol[:], bnum[:],
                             start=True, stop=True)
            arow = pp.tile([1, 520], f32, tag="arow")
            nc.vector.memset(arow[:], 0.0)
            nc.vector.tensor_copy(arow[0:1, 0:D2], hnum_sb[:])
            nc.vector.tensor_reduce(arow[0:1, D2:D2 + 1], ssps[:], X, ADD)
            ar_in = dram.tile([1, 520], f32, tag="ar_in")
            ar_out = dram.tile([1, 520], f32, tag="ar_out")
            nc.sync.dma_start(ar_in[:], arow[:])
            nc.gpsimd.collective_compute(
                "AllReduce", ADD, replica_groups=[list(range(NCORES))],
                ins=[ar_in.opt()], outs=[ar_out.opt()],
            )
            hg = pp.tile([1, 520], f32, tag="hg")
            nc.sync.dma_start(hg[:], ar_out[:])

            # h~ = hnum_g / ssum_g, broadcast to all partitions
            zinv = pp.tile([1, 1], f32, tag="zinv")
            nc.vector.reciprocal(zinv[:], hg[0:1, D2:D2 + 1])
            htrow = pp.tile([1, D2], f32r, tag="htrow")
            nc.vector.tensor_scalar_mul(htrow[:], hg[0:1, 0:D2], zinv[:])
            htps = rowpsum.tile([128, D2], f32, tag="bcast")
            nc.tensor.matmul(htps[:], onesrow[:], htrow[:],
                             start=True, stop=True)

            # G block 3: H * h~
            for tt in range(NTT):
                hh_sb = gp.tile([128, D2], f32, tag="hh_sb")
                nc.vector.tensor_tensor(hh_sb[:], f(hn[:, tt, :]), htps[:], MULT)
                nc.sync.dma_start(G.ap()[tt * 128:(tt + 1) * 128, 3 * D2:4 * D2],
                                  hh_sb[:])

    nc.compile()
    return nc


def kernel(H, U, Ws):
    import concourse.mybir as mybir
    from concourse import bass_utils

    H = np.ascontiguousarray(np.asarray(H, dtype=np.float32))
    U = np.ascontiguousarray(np.asarray(U, dtype=np.float32))
    Ws = np.asarray(Ws, dtype=np.float32)

    if "nc" not in _CACHE:
        _CACHE["nc"] = _build_nc()
    nc = _CACHE["nc"]

    mmnp = (mybir.dt.np(mybir.dt.bfloat16) if MM_BF16 else np.float32)

    UT = np.ascontiguousarray(U.T)
    w3 = Ws[2 * D2:3 * D2, 0]
    UW = np.ascontiguousarray(UT * w3[:, None]).astype(mmnp)
    Wc = np.ascontiguousarray(Ws.reshape(3, D2).T[:, 0:2]).astype(mmnp)
    UTc = UT.astype(mmnp)
    Unc = U.astype(mmnp)
    ident = np.eye(128, dtype=np.float32)

    in_maps = []
    for c in range(NCORES):
        Hc = H[c * TC:(c + 1) * TC]
        in_maps.append({
            "HT": np.ascontiguousarray(Hc.T).astype(mmnp),
            "Hn": Hc,
            "UT": UTc,
            "Un": Unc,
            "UW": UW,
            "Wc": Wc,
            "Id": ident,
            "On": np.ones((1, 128), dtype=np.float32),
            "Oc": np.ones((128, 1), dtype=np.float32),
        })

    res = bass_utils.run_bass_kernel_spmd(
        nc, in_maps, core_ids=list(range(NCORES)))
    LAST["exec_time_ns"] = res.exec_time_ns
    G_full = np.concatenate([res.results[c]["G"] for c in range(NCORES)],
                            axis=0)
    return G_full.astype(np.float32, copy=False)
